# revision 1
# baseline (speedup 1.0000x reference)
"""Block-diagonal linear for Trainium2 (8 NeuronCores, batch-data-parallel).

y[b,c,o] = sum_i x[b,c,i]*W[c,o,i] + bias[c,o], x [16384, 3072] f32.
Sharding: batch split 8 ways (2048 rows/core); W/bias replicated, pre-reshaped
host-side into fp16 weight-image rows (i-major) broadcast across partitions,
staged as two DMAs so the first multiply starts early.

Per fused group of 1-2 128-row tiles (small first/last groups cut pipeline
fill/drain): SWDGE cast-DMA in (f32->fp16); ScalarE deinterleaves per-i;
DVE does 3 wide muls (broadcast over o) + 2 wide adds + 3 per-o bias-adds,
all fp16 2x mode; ScalarE interleaves per-o; SWDGE cast-DMA out (fp16->f32).
"""

import numpy as np

import concourse.bacc as bacc
import concourse.mybir as mybir
from concourse import bass_utils
from concourse.tile import TileContext

N_CORES = 8
B_FULL = 16384
F = 3072
C = F // 3  # 1024
B_CORE = B_FULL // N_CORES  # 2048
P = 128
GROUPS = [1, 1] + [2] * 6 + [1, 1]  # tiles per fused group (sum = 16)
FP32 = mybir.dt.float32
FP16 = mybir.dt.float16


def build_bass():
    nc = bacc.Bacc("TRN2", num_devices=N_CORES)
    x = nc.dram_tensor("x", [B_CORE, F], FP32, kind="ExternalInput")
    wba = nc.dram_tensor("wb16a", [P, 3 * C], FP16, kind="ExternalInput")
    wbb = nc.dram_tensor("wb16b", [P, 9 * C], FP16, kind="ExternalInput")
    y = nc.dram_tensor("y", [B_CORE, F], FP32, kind="ExternalOutput")

    with TileContext(nc) as tc:
        with (
            tc.tile_pool(name="wpool", bufs=1) as wpool,
            tc.tile_pool(name="xpool", bufs=2) as xpool,
            tc.tile_pool(name="ypool", bufs=2) as ypool,
            tc.tile_pool(name="xdpool", bufs=2) as xdpool,
            tc.tile_pool(name="ydpool", bufs=2) as ydpool,
            tc.tile_pool(name="tpool", bufs=2) as tpool,
        ):
            wba_sb = wpool.tile([P, 3 * C], FP16)
            wbb_sb = wpool.tile([P, 9 * C], FP16)
            # o=0 weight images first on the SWDGE FIFO so the o=0 chain
            # can start early; the rest lands between the first x loads
            nc.gpsimd.dma_start(out=wba_sb[:, :], in_=wba.ap()[:, :])

            # i-major: wba = i=0 images [o, c]; wbb = i=1,2 images + bias
            def wslice(i):
                if i == 0:
                    return wba_sb[:, :]
                return wbb_sb[:, (i - 1) * 3 * C : i * 3 * C]

            wimg = lambda i, gt: (
                wslice(i)
                .rearrange("p (o c) -> p o c", o=3)
                .unsqueeze(2)
                .broadcast_to([P, 3, gt, C])
            )
            bimg = lambda o, gt: (
                wbb_sb[:, (6 + o) * C : (7 + o) * C]
                .unsqueeze(1)
                .broadcast_to([P, gt, C])
            )
            probe = wpool.tile([P, 1], FP16)
            nc.vector.tensor_copy(out=probe[:, :], in_=wba_sb[:, :1])
            probe2 = wpool.tile([P, 1], FP16)
            nc.scalar.copy(probe2[:, :], wba_sb[:, :1])

            tile0 = 0
            for g, gt in enumerate(GROUPS):
                r0 = tile0 * P
                tile0 += gt
                x16 = xpool.tile([P, gt * F], FP16, tag="x", name=f"x16_{g}")
                y16 = ypool.tile([P, gt * F], FP16, tag="y", name=f"y16_{g}")
                xdram = x.ap()[r0 : r0 + gt * P, :].rearrange(
                    "(t p) f -> p t f", p=P
                )
                ydram = y.ap()[r0 : r0 + gt * P, :].rearrange(
                    "(t p) f -> p t f", p=P
                )
                # cast-DMA in (SWDGE): [p, t, f]
                nc.gpsimd.dma_start(
                    out=x16[:, :].rearrange("p (t f) -> p t f", f=F),
                    in_=xdram,
                )
                if g == 0:
                    nc.gpsimd.dma_start(out=wbb_sb[:, :], in_=wbb.ap()[:, :])
                # [p, t, c, i] view
                x4 = x16[:, :].rearrange(
                    "p (t c three) -> p t c three", t=gt, three=3
                )
                y4 = y16[:, :].rearrange(
                    "p (t c three) -> p t c three", t=gt, three=3
                )

                xd = [
                    xdpool.tile([P, gt * C], FP16, tag=f"xd{i}", name=f"xd{i}_{g}")
                    for i in range(3)
                ]
                for i in range(3):
                    nc.scalar.copy(
                        xd[i][:, :].rearrange("p (t c) -> p t c", c=C),
                        x4[:, :, :, i],
                    )

                acc = tpool.tile([P, 3 * gt * C], FP16, tag="acc", name=f"acc_{g}")
                tmp = tpool.tile([P, 3 * gt * C], FP16, tag="tmp", name=f"tmp_{g}")
                yd = ydpool.tile([P, 3 * gt * C], FP16, tag="yd", name=f"yd_{g}")
                a4 = acc[:, :].rearrange("p (o t c) -> p o t c", o=3, t=gt)
                t4 = tmp[:, :].rearrange("p (o t c) -> p o t c", o=3, t=gt)
                yd4 = yd[:, :].rearrange("p (o t c) -> p o t c", o=3, t=gt)
                xin = lambda i: (
                    xd[i][:, :]
                    .rearrange("p (t c) -> p t c", c=C)
                    .unsqueeze(1)
                    .broadcast_to([P, 3, gt, C])
                )
                nc.vector.tensor_mul(a4, xin(0), wimg(0, gt))
                nc.vector.tensor_mul(t4, xin(1), wimg(1, gt))
                nc.vector.tensor_add(acc[:, :], acc[:, :], tmp[:, :])
                nc.vector.tensor_mul(t4, xin(2), wimg(2, gt))
                nc.vector.tensor_add(acc[:, :], acc[:, :], tmp[:, :])
                for o in range(3):
                    nc.vector.tensor_add(yd4[:, o], a4[:, o], bimg(o, gt))
                    nc.scalar.copy(y4[:, :, :, o], yd4[:, o])

                # cast-DMA out (SWDGE)
                nc.gpsimd.dma_start(
                    out=ydram,
                    in_=y16[:, :].rearrange("p (t f) -> p t f", f=F),
                )

    nc.compile()
    return nc


def _prep_small(W, b):
    wimg = W.transpose(2, 1, 0).reshape(9 * C)  # [i, o, c] i-major
    bimg = b.T.reshape(3 * C)
    wa = wimg[: 3 * C].astype(np.float16)  # i=0 images
    wbv = np.concatenate([wimg[3 * C :], bimg]).astype(np.float16)
    return (
        np.ascontiguousarray(np.broadcast_to(wa, (P, 3 * C))),
        np.ascontiguousarray(np.broadcast_to(wbv, (P, 9 * C))),
    )


def run(x, W, b, trace=False, **run_kwargs):
    nc = build_bass()
    wa, wbv = _prep_small(np.asarray(W), np.asarray(b))
    x = np.asarray(x, dtype=np.float32)
    in_maps = [
        {
            "x": np.ascontiguousarray(x[k * B_CORE : (k + 1) * B_CORE]),
            "wb16a": wa,
            "wb16b": wbv,
        }
        for k in range(N_CORES)
    ]
    res = bass_utils.run_bass_kernel_spmd(
        nc, in_maps, core_ids=list(range(N_CORES)), trace=trace, **run_kwargs
    )
    y = np.concatenate([r["y"] for r in res.results], axis=0)
    return y, res


def kernel(x, W, b):
    y, _ = run(x, W, b, trace=False)
    return y



# revision 2
# speedup vs baseline: 1.0119x; 1.0119x over previous
"""Block-diagonal linear for Trainium2 (8 NeuronCores, batch-data-parallel).

y[b,c,o] = sum_i x[b,c,i]*W[c,o,i] + bias[c,o], x [16384, 3072] f32.

v3: DVE computes 3 partial products (+bias in p0) as fp16 2x tensor_tensor;
TensorE sums them in PSUM with identity-matmul copies (f32 accumulate),
N=512 full-bank matmuls with contiguous rhs so the PE streams at line rate
and stays busy enough to clock up. PSUM tile [P, 3*512] holds (o-major)
one c-half of a row-tile; ScalarE drains it with a strided (c,o)-interleave
read and a contiguous fp16 write. SWDGE cast-DMAs both ways.
"""

import numpy as np

import concourse.bacc as bacc
import concourse.mybir as mybir
from concourse import bass_utils
from concourse.tile import TileContext

N_CORES = 8
B_FULL = 16384
F = 3072
C = F // 3  # 1024
B_CORE = B_FULL // N_CORES  # 2048
P = 128
GROUPS = [1, 1] + [2] * 6 + [1, 1]  # tiles per fused group (sum = 16)
CH = 512  # c's per psum chunk (half of C)
NH = C // CH  # 2 chunks per row-tile
FP32 = mybir.dt.float32
FP16 = mybir.dt.float16


def build_bass():
    nc = bacc.Bacc("TRN2", num_devices=N_CORES)
    x = nc.dram_tensor("x", [B_CORE, F], FP32, kind="ExternalInput")
    wimg = nc.dram_tensor("wimg", [P, 9 * C], FP16, kind="ExternalInput")
    bimg = nc.dram_tensor("bimg", [P, 3 * C], FP16, kind="ExternalInput")
    eye = nc.dram_tensor("eye", [P, P], FP16, kind="ExternalInput")
    y = nc.dram_tensor("y", [B_CORE, F], FP32, kind="ExternalOutput")

    with TileContext(nc) as tc:
        with (
            tc.tile_pool(name="wpool", bufs=1) as wpool,
            tc.tile_pool(name="xpool", bufs=2) as xpool,
            tc.tile_pool(name="xdpool", bufs=2) as xdpool,
            tc.tile_pool(name="ppool", bufs=2) as ppool,
            tc.tile_pool(name="ypool", bufs=2) as ypool,
            tc.psum_pool(name="psum", bufs=2) as psum_pool,
        ):
            w_sb = wpool.tile([P, 9 * C], FP16)
            b_sb = wpool.tile([P, 3 * C], FP16)
            eye_sb = wpool.tile([P, P], FP16)
            nc.sync.dma_start(out=eye_sb[:, :], in_=eye.ap()[:, :])
            nc.sync.dma_start(out=w_sb[:, :], in_=wimg.ap()[:, :])
            nc.sync.dma_start(out=b_sb[:, :], in_=bimg.ap()[:, :])

            # w image i: [o, c] layout (o major), broadcast over t
            def wview(i, gt):
                return (
                    w_sb[:, i * 3 * C : (i + 1) * 3 * C]
                    .rearrange("p (o c) -> p o c", o=3)
                    .unsqueeze(2)
                    .broadcast_to([P, 3, gt, C])
                )

            def bview(gt):
                return (
                    b_sb[:, :]
                    .rearrange("p (o c) -> p o c", o=3)
                    .unsqueeze(2)
                    .broadcast_to([P, 3, gt, C])
                )

            tile0 = 0
            for g, gt in enumerate(GROUPS):
                r0 = tile0 * P
                tile0 += gt
                x16 = xpool.tile([P, gt * F], FP16, tag="x", name=f"x16_{g}")
                y16 = ypool.tile([P, gt * F], FP16, tag="y", name=f"y16_{g}")
                xdram = x.ap()[r0 : r0 + gt * P, :].rearrange(
                    "(t p) f -> p t f", p=P
                )
                ydram = y.ap()[r0 : r0 + gt * P, :].rearrange(
                    "(t p) f -> p t f", p=P
                )
                # cast-DMA in (SWDGE): f32 HBM -> fp16 SBUF
                nc.gpsimd.dma_start(
                    out=x16[:, :].rearrange("p (t f) -> p t f", f=F),
                    in_=xdram,
                )
                x4 = x16[:, :].rearrange(
                    "p (t c three) -> p t c three", t=gt, three=3
                )

                # ScalarE: deinterleave + per-i contiguous xd
                xd = [
                    xdpool.tile([P, gt * C], FP16, tag=f"xd{i}", name=f"xd{i}_{g}")
                    for i in range(3)
                ]
                for i in range(3):
                    nc.scalar.copy(
                        xd[i][:, :].rearrange("p (t c) -> p t c", c=C),
                        x4[:, :, :, i],
                    )

                xin = lambda i: (
                    xd[i][:, :]
                    .rearrange("p (t c) -> p t c", c=C)
                    .unsqueeze(1)
                    .broadcast_to([P, 3, gt, C])
                )

                # DVE: 3 partial products (o-major layout), bias folded into p0
                pt = [
                    ppool.tile([P, 3 * gt * C], FP16, tag=f"p{i}", name=f"p{i}_{g}")
                    for i in range(3)
                ]
                pv = [
                    pt[i][:, :].rearrange("p (o t c) -> p o t c", o=3, t=gt)
                    for i in range(3)
                ]
                nc.vector.tensor_mul(pv[0], xin(0), wview(0, gt))
                nc.vector.tensor_add(pv[0], pv[0], bview(gt))
                nc.vector.tensor_mul(pv[1], xin(1), wview(1, gt))
                nc.vector.tensor_mul(pv[2], xin(2), wview(2, gt))

                # TensorE: identity-matmul copies sum p0+p1+p2 into PSUM in
                # f32.  One psum tile [P, 3*CH] = 3 banks holds all o's of a
                # c-half; each matmul (N=512 contiguous rhs) fills one bank.
                yt = y16[:, :].rearrange(
                    "p (t c three) -> p t c three", t=gt, three=3
                )
                for t in range(gt):
                    for h in range(NH):
                        ps = psum_pool.tile(
                            [P, 3 * CH], FP32, tag="ps", name=f"ps_{g}_{t}_{h}"
                        )
                        for o in range(3):
                            for i in range(3):
                                nc.tensor.matmul(
                                    out=ps[:, o * CH : (o + 1) * CH],
                                    lhsT=eye_sb[:, :],
                                    rhs=pv[i][:, o, t, h * CH : (h + 1) * CH],
                                    start=(i == 0),
                                    stop=(i == 2),
                                )
                        # drain: strided (c,o)-interleave read from PSUM,
                        # contiguous fp16 write into the output tile
                        nc.scalar.copy(
                            yt[:, t, h * CH : (h + 1) * CH, :],
                            ps[:, :].rearrange("p (o c) -> p c o", o=3),
                        )

                # cast-DMA out (SWDGE): fp16 SBUF -> f32 HBM
                nc.gpsimd.dma_start(
                    out=ydram,
                    in_=y16[:, :].rearrange("p (t f) -> p t f", f=F),
                )

    nc.compile()
    return nc


def _prep_small(W, b):
    # w image i, flattened [o, c] o-major: wimg[i, o*C + c] = W[c, o, i]
    wflat = W.transpose(2, 1, 0).reshape(9 * C).astype(np.float16)  # [i, o, c]
    bflat = b.T.reshape(3 * C).astype(np.float16)  # [o, c]
    eye = np.eye(P, dtype=np.float16)
    return (
        np.ascontiguousarray(np.broadcast_to(wflat, (P, 9 * C))),
        np.ascontiguousarray(np.broadcast_to(bflat, (P, 3 * C))),
        eye,
    )


def run(x, W, b, trace=False, **run_kwargs):
    nc = build_bass()
    wa, ba, eye = _prep_small(np.asarray(W), np.asarray(b))
    x = np.asarray(x, dtype=np.float32)
    in_maps = [
        {
            "x": np.ascontiguousarray(x[k * B_CORE : (k + 1) * B_CORE]),
            "wimg": wa,
            "bimg": ba,
            "eye": eye,
        }
        for k in range(N_CORES)
    ]
    res = bass_utils.run_bass_kernel_spmd(
        nc, in_maps, core_ids=list(range(N_CORES)), trace=trace, **run_kwargs
    )
    y = np.concatenate([r["y"] for r in res.results], axis=0)
    return y, res


def kernel(x, W, b):
    y, _ = run(x, W, b, trace=False)
    return y


# revision 3
# speedup vs baseline: 1.0418x; 1.0295x over previous
"""Block-diagonal linear for Trainium2 (8 NeuronCores, batch-data-parallel).

y[b,c,o] = sum_i x[b,c,i]*W[c,o,i] + bias[c,o], x [16384, 3072] f32.

v3: DVE computes 3 partial products (+bias in p0) as fp16 2x tensor_tensor;
TensorE sums them in PSUM with identity-matmul copies (f32 accumulate),
N=256 matmuls (psum tiles 2 banks x 4 bufs keep the PE p-state high) with contiguous rhs so the PE streams at line rate
and stays busy enough to clock up. PSUM tile [P, 3*512] holds (o-major)
one c-half of a row-tile; ScalarE drains it with a strided (c,o)-interleave
read and a contiguous fp16 write. SWDGE cast-DMAs both ways.
"""

import numpy as np

import concourse.bacc as bacc
import concourse.mybir as mybir
from concourse import bass_utils
from concourse.tile import TileContext

N_CORES = 8
B_FULL = 16384
F = 3072
C = F // 3  # 1024
B_CORE = B_FULL // N_CORES  # 2048
P = 128
GROUPS = [1, 1] + [2] * 6 + [1, 1]  # tiles per fused group (sum = 16)
CH = 256  # c's per psum chunk
NH = C // CH  # 4 chunks per row-tile
FP32 = mybir.dt.float32
FP16 = mybir.dt.float16


def build_bass():
    nc = bacc.Bacc("TRN2", num_devices=N_CORES)
    x = nc.dram_tensor("x", [B_CORE, F], FP32, kind="ExternalInput")
    wimg = nc.dram_tensor("wimg", [P, 9 * C], FP16, kind="ExternalInput")
    bimg = nc.dram_tensor("bimg", [P, 3 * C], FP16, kind="ExternalInput")
    eye = nc.dram_tensor("eye", [P, P], FP16, kind="ExternalInput")
    y = nc.dram_tensor("y", [B_CORE, F], FP32, kind="ExternalOutput")

    with TileContext(nc) as tc:
        with (
            tc.tile_pool(name="wpool", bufs=1) as wpool,
            tc.tile_pool(name="xpool", bufs=2) as xpool,
            tc.tile_pool(name="xdpool", bufs=2) as xdpool,
            tc.tile_pool(name="ppool", bufs=2) as ppool,
            tc.tile_pool(name="ypool", bufs=2) as ypool,
            tc.psum_pool(name="psum", bufs=4) as psum_pool,
        ):
            w_sb = wpool.tile([P, 9 * C], FP16)
            b_sb = wpool.tile([P, 3 * C], FP16)
            eye_sb = wpool.tile([P, P], FP16)
            nc.sync.dma_start(out=eye_sb[:, :], in_=eye.ap()[:, :])
            nc.sync.dma_start(out=w_sb[:, :], in_=wimg.ap()[:, :])
            nc.sync.dma_start(out=b_sb[:, :], in_=bimg.ap()[:, :])

            # w image i: [o, c] layout (o major), broadcast over t
            def wview(i, gt):
                return (
                    w_sb[:, i * 3 * C : (i + 1) * 3 * C]
                    .rearrange("p (o c) -> p o c", o=3)
                    .unsqueeze(2)
                    .broadcast_to([P, 3, gt, C])
                )

            def bview(gt):
                return (
                    b_sb[:, :]
                    .rearrange("p (o c) -> p o c", o=3)
                    .unsqueeze(2)
                    .broadcast_to([P, 3, gt, C])
                )

            tile0 = 0
            for g, gt in enumerate(GROUPS):
                r0 = tile0 * P
                tile0 += gt
                x16 = xpool.tile([P, gt * F], FP16, tag="x", name=f"x16_{g}")
                y16 = ypool.tile([P, gt * F], FP16, tag="y", name=f"y16_{g}")
                xdram = x.ap()[r0 : r0 + gt * P, :].rearrange(
                    "(t p) f -> p t f", p=P
                )
                ydram = y.ap()[r0 : r0 + gt * P, :].rearrange(
                    "(t p) f -> p t f", p=P
                )
                # cast-DMA in (SWDGE): f32 HBM -> fp16 SBUF
                nc.gpsimd.dma_start(
                    out=x16[:, :].rearrange("p (t f) -> p t f", f=F),
                    in_=xdram,
                )
                x4 = x16[:, :].rearrange(
                    "p (t c three) -> p t c three", t=gt, three=3
                )

                # ScalarE: deinterleave + per-i contiguous xd
                xd = [
                    xdpool.tile([P, gt * C], FP16, tag=f"xd{i}", name=f"xd{i}_{g}")
                    for i in range(3)
                ]
                for i in range(3):
                    nc.scalar.copy(
                        xd[i][:, :].rearrange("p (t c) -> p t c", c=C),
                        x4[:, :, :, i],
                    )

                xin = lambda i: (
                    xd[i][:, :]
                    .rearrange("p (t c) -> p t c", c=C)
                    .unsqueeze(1)
                    .broadcast_to([P, 3, gt, C])
                )

                # DVE: 3 partial products (o-major layout), bias folded into p0
                pt = [
                    ppool.tile([P, 3 * gt * C], FP16, tag=f"p{i}", name=f"p{i}_{g}")
                    for i in range(3)
                ]
                pv = [
                    pt[i][:, :].rearrange("p (o t c) -> p o t c", o=3, t=gt)
                    for i in range(3)
                ]
                nc.vector.tensor_mul(pv[0], xin(0), wview(0, gt))
                nc.vector.tensor_add(pv[0], pv[0], bview(gt))
                nc.vector.tensor_mul(pv[1], xin(1), wview(1, gt))
                nc.vector.tensor_mul(pv[2], xin(2), wview(2, gt))

                # TensorE: identity-matmul copies sum p0+p1+p2 into PSUM in
                # f32.  One psum tile [P, 3*CH] = 3 banks holds all o's of a
                # c-half; each matmul (N=512 contiguous rhs) fills one bank.
                yt = y16[:, :].rearrange(
                    "p (t c three) -> p t c three", t=gt, three=3
                )
                for t in range(gt):
                    for h in range(NH):
                        ps = psum_pool.tile(
                            [P, 3 * CH], FP32, tag="ps", name=f"ps_{g}_{t}_{h}"
                        )
                        for o in range(3):
                            for i in range(3):
                                nc.tensor.matmul(
                                    out=ps[:, o * CH : (o + 1) * CH],
                                    lhsT=eye_sb[:, :],
                                    rhs=pv[i][:, o, t, h * CH : (h + 1) * CH],
                                    start=(i == 0),
                                    stop=(i == 2),
                                )
                        # drain: strided (c,o)-interleave read from PSUM,
                        # contiguous fp16 write into the output tile
                        nc.scalar.copy(
                            yt[:, t, h * CH : (h + 1) * CH, :],
                            ps[:, :].rearrange("p (o c) -> p c o", o=3),
                        )

                # cast-DMA out (SWDGE): fp16 SBUF -> f32 HBM
                nc.gpsimd.dma_start(
                    out=ydram,
                    in_=y16[:, :].rearrange("p (t f) -> p t f", f=F),
                )

    nc.compile()
    return nc


def _prep_small(W, b):
    # w image i, flattened [o, c] o-major: wimg[i, o*C + c] = W[c, o, i]
    wflat = W.transpose(2, 1, 0).reshape(9 * C).astype(np.float16)  # [i, o, c]
    bflat = b.T.reshape(3 * C).astype(np.float16)  # [o, c]
    eye = np.eye(P, dtype=np.float16)
    return (
        np.ascontiguousarray(np.broadcast_to(wflat, (P, 9 * C))),
        np.ascontiguousarray(np.broadcast_to(bflat, (P, 3 * C))),
        eye,
    )


def run(x, W, b, trace=False, **run_kwargs):
    nc = build_bass()
    wa, ba, eye = _prep_small(np.asarray(W), np.asarray(b))
    x = np.asarray(x, dtype=np.float32)
    in_maps = [
        {
            "x": np.ascontiguousarray(x[k * B_CORE : (k + 1) * B_CORE]),
            "wimg": wa,
            "bimg": ba,
            "eye": eye,
        }
        for k in range(N_CORES)
    ]
    res = bass_utils.run_bass_kernel_spmd(
        nc, in_maps, core_ids=list(range(N_CORES)), trace=trace, **run_kwargs
    )
    y = np.concatenate([r["y"] for r in res.results], axis=0)
    return y, res


def kernel(x, W, b):
    y, _ = run(x, W, b, trace=False)
    return y
